# revision 2
# baseline (speedup 1.0000x reference)
"""2-layer relational GCN (RGCN) on Trainium2, 8-core SPMD — v3.

Sharding: edges partitioned by dst-node range (core c owns dst nodes
[c*S, (c+1)*S)); node features and weights replicated. Self-loops folded
in as relation R.

Single-phase layer structure (no message buffer round trip):
  Slots ordered (src-half, dst-tile, rel). One SWDGE dma_gather stream
  per src-half (int16 index limit) pulls node rows (bf16, 256B) with
  fused transpose, directly usable as matmul lhsT columns. Each
  128-slot chunk belongs to ONE dst tile; its relation segments are
  matmul'd (partition-range PSUM writes) with the per-relation weights,
  staged to bf16, and immediately aggregated into the tile's PSUM
  accumulator with a one-hot dst-selection matmul. The lo-half pass
  parks per-tile partials in SBUF; the hi-half pass adds them back,
  applies bias (+ReLU layer 1), and writes the shard.
  AllGather (bf16, 128-padded h rows) between layers; both layers share
  the same slot order and index tables.

Slot-space structure (group sizes, chunk maps, segment lists, op lists)
is the max over cores so all 8 cores run one program; per-core tables
embed that core's edges (pad slots: gather idx 0, one-hot d = -1).
All SWDGE ops stay on queue 0: concurrent multi-queue SWDGE gathers
lose data on this hardware (empirically; the ring serialization when
ops exceed half the 1024-descriptor carveout is the only stable mode).
"""

import numpy as np

P = 128
C = 8
HALF = 32768
GOP = 896         # slots per gather op (>=1024-desc ops hang the DGE ring)
SCH = 8           # chunks per PSUM bank batch
BUFS = 4
NO_COLLECTIVE = False

_CACHE = {}


def _wrap16(a):
    a = np.asarray(a, np.int16)
    assert len(a) % 16 == 0
    w = a.reshape(-1, 16).T
    return np.ascontiguousarray(np.tile(w, (8, 1)))


# ---------------------------------------------------------------- host prep

def _preprocess(feat, W1, loop1, b1, W2, loop2, b2, src, dst, etype):
    import ml_dtypes

    feat = np.asarray(feat, dtype=np.float32)
    W1 = np.asarray(W1, dtype=np.float32)
    W2 = np.asarray(W2, dtype=np.float32)
    loop1 = np.asarray(loop1, dtype=np.float32)
    loop2 = np.asarray(loop2, dtype=np.float32)
    b1 = np.asarray(b1, dtype=np.float32)
    b2 = np.asarray(b2, dtype=np.float32)
    src = np.asarray(src).astype(np.int64).ravel()
    dst = np.asarray(dst).astype(np.int64).ravel()
    etype = np.asarray(etype).astype(np.int64).ravel()

    N, D = feat.shape
    R, _, H = W1.shape
    O = W2.shape[2]
    assert D == P and N % C == 0
    S = N // C
    NT = -(-S // P)
    Rp = R + 1

    sl = np.arange(N, dtype=np.int64)
    asrc = np.concatenate([src, sl])
    adst = np.concatenate([dst, sl])
    aet = np.concatenate([etype, np.full(N, R, dtype=np.int64)])
    core_of = adst // S

    cores = []
    for c in range(C):
        m = core_of == c
        es, el, er = asrc[m], adst[m] - c * S, aet[m]
        cores.append((es, el, er, el // P, (es >= HALF).astype(np.int64)))

    # common per-(half, tile, rel) max counts, padded to 32 so relation
    # segment boundaries land on legal matmul partition offsets
    cnt = np.zeros((2, NT, Rp), np.int64)
    for es, el, er, tid, hf in cores:
        k = (hf * NT + tid) * Rp + er
        cnt = np.maximum(
            cnt, np.bincount(k, minlength=2 * NT * Rp).reshape(2, NT, Rp))
    cnt = ((cnt + 31) // 32) * 32

    # slot layout: (half, tile, rel) groups unpadded; (half, tile) blocks
    # padded to 128; both halves concatenated
    ht_size = cnt.sum(axis=2)                        # [2, NT]
    ht_pad = np.maximum(((ht_size + P - 1) // P) * P, P)
    ht_off = np.zeros((2, NT + 1), np.int64)
    base = 0
    half_range = []
    for h in range(2):
        h0 = base
        for t in range(NT):
            ht_off[h, t] = base
            base += int(ht_pad[h, t])
        ht_off[h, NT] = base
        half_range.append((h0, base))
    T3 = base // P
    assert base % P == 0

    # group offsets within each (h, t) block
    goff = np.zeros((2, NT, Rp + 1), np.int64)
    for h in range(2):
        for t in range(NT):
            goff[h, t, 1:] = np.cumsum(cnt[h, t])
            goff[h, t] += ht_off[h, t]

    # chunk maps + per-chunk relation segments
    chunk_tile = np.zeros(T3, np.int64)
    chunk_half = np.zeros(T3, np.int64)
    chunk_k = np.zeros(T3, np.int64)
    chunk_kt = np.zeros(T3, np.int64)
    chunk_segs = []
    for h in range(2):
        for t in range(NT):
            kt = int(ht_pad[h, t]) // P
            c0 = int(ht_off[h, t]) // P
            for k in range(kt):
                gc = c0 + k
                chunk_tile[gc] = t
                chunk_half[gc] = h
                chunk_k[gc] = k
                chunk_kt[gc] = kt
                lo = ht_off[h, t] + k * P
                hi = lo + P
                segs = []
                for r in range(Rp):
                    a = max(int(goff[h, t, r]), int(lo))
                    b = min(int(goff[h, t, r + 1]), int(hi))
                    if b > a:
                        segs.append((int(a - lo), int(b - lo), r))
                if not segs:
                    segs = [(0, P, 0)]
                else:
                    a0, _, r0 = segs[0]
                    if a0 > 0:
                        segs[0] = (0, segs[0][1], r0)
                    an, bn, rn = segs[-1]
                    if bn < P:
                        segs[-1] = (an, P, rn)
                # matmul PSUM partition windows are limited to [0,*),
                # [32,64), [64,*): emit segments high-to-low, each from the
                # largest legal base covering its span; clobbered low rows
                # are rewritten by the following (lower) segments
                segs = sorted(segs, key=lambda x: -x[0])
                fixed = []
                for (a, b, r) in segs:
                    if b <= 64:
                        base_p = 32 if a >= 32 else 0
                    else:
                        base_p = 64 if a >= 64 else 0
                    fixed.append((base_p, b, r))
                chunk_segs.append(tuple(fixed))

    # gather ops: per half, 128-aligned, <= GOP slots
    gops = []
    for h in range(2):
        a, b = half_range[h]
        s0 = a
        while s0 < b:
            n = min(GOP, b - s0)
            gops.append((int(s0), int(n), h))
            s0 += n

    # replicated tensors
    feat_bf = np.ascontiguousarray(feat.astype(ml_dtypes.bfloat16))
    w1f = np.ascontiguousarray(
        np.concatenate([W1, loop1[None]], axis=0)
        .transpose(1, 0, 2).reshape(D, Rp * H).astype(ml_dtypes.bfloat16))
    W2e = np.concatenate([W2, loop2[None]], axis=0)
    w2p = np.zeros((P, Rp * O), np.float32)
    w2p[:H, :] = W2e.transpose(1, 0, 2).reshape(H, Rp * O)
    w2f = np.ascontiguousarray(w2p.astype(ml_dtypes.bfloat16))
    b1b = np.ascontiguousarray(np.broadcast_to(b1, (P, H)).astype(np.float32))
    b2b = np.ascontiguousarray(np.broadcast_to(b2, (P, O)).astype(np.float32))

    # per-core tables
    in_maps = []
    for es, el, er, tid, hf in cores:
        nE = len(es)
        gkey = (hf * NT + tid) * Rp + er
        order = np.lexsort((es, gkey))
        sk = gkey[order]
        uq, ust = np.unique(sk, return_index=True)
        goff_flat = goff[:, :, :Rp].reshape(-1)
        slot_sorted = (goff_flat[sk]
                       + (np.arange(nE) - ust[np.searchsorted(uq, sk)]))
        slot = np.empty(nE, np.int64)
        slot[order] = slot_sorted

        gidx = np.zeros(T3 * P, np.int16)
        gidx[slot] = (es - HALF * hf).astype(np.int16)
        d3 = np.full(T3 * P, -1.0, np.float32)
        d3[slot] = (el % P).astype(np.float32)

        in_maps.append({
            "feat_bf": feat_bf, "w1f": w1f, "w2f": w2f,
            "b1b": b1b, "b2b": b2b,
            "gidx": _wrap16(gidx),
            "d3t": np.ascontiguousarray(d3.reshape(T3, P).T),
        })

    plan = dict(N=N, D=D, H=H, O=O, Rp=Rp, S=S, NT=NT, T3=T3,
                gops=tuple(gops),
                chunk_tile=tuple(int(x) for x in chunk_tile),
                chunk_half=tuple(int(x) for x in chunk_half),
                chunk_k=tuple(int(x) for x in chunk_k),
                chunk_kt=tuple(int(x) for x in chunk_kt),
                chunk_segs=tuple(chunk_segs))
    return plan, in_maps


# ---------------------------------------------------------------- device prog

def _bc_inner(ap, n):
    import concourse.bass as bass
    return bass.AP(ap.tensor, ap.offset, list(ap.ap) + [[0, n]])


def _bc_mid(ap, g):
    import concourse.bass as bass
    a = list(ap.ap)
    return bass.AP(ap.tensor, ap.offset, [a[0], [0, g]] + a[1:])


def _build(plan):
    import concourse.bacc as bacc
    import concourse.tile as tile
    import concourse.mybir as mybir

    N, D, H, O, Rp = plan["N"], plan["D"], plan["H"], plan["O"], plan["Rp"]
    S, NT, T3 = plan["S"], plan["NT"], plan["T3"]
    gops = plan["gops"]
    chunk_tile, chunk_half = plan["chunk_tile"], plan["chunk_half"]
    chunk_k, chunk_kt = plan["chunk_k"], plan["chunk_kt"]
    chunk_segs = plan["chunk_segs"]
    f32 = mybir.dt.float32
    bf16 = mybir.dt.bfloat16
    i16 = mybir.dt.int16
    i32 = mybir.dt.int32
    AO = mybir.AluOpType

    nc = bacc.Bacc("TRN2", target_bir_lowering=False, debug=False,
                   num_devices=C)
    feat_bf = nc.dram_tensor("feat_bf", [N, P], bf16, kind="ExternalInput")
    w1f = nc.dram_tensor("w1f", [P, Rp * H], bf16, kind="ExternalInput")
    w2f = nc.dram_tensor("w2f", [P, Rp * O], bf16, kind="ExternalInput")
    b1b = nc.dram_tensor("b1b", [P, H], f32, kind="ExternalInput")
    b2b = nc.dram_tensor("b2b", [P, O], f32, kind="ExternalInput")
    gidxt = nc.dram_tensor("gidx", [P, T3 * P // 16], i16,
                           kind="ExternalInput")
    d3t = nc.dram_tensor("d3t", [P, T3], f32, kind="ExternalInput")
    outs = nc.dram_tensor("out_shard", [S, O], f32, kind="ExternalOutput")

    with tile.TileContext(nc) as tc:
        with tc.tile_pool(name="dram", bufs=1, space="DRAM") as dramp:
            h_shard = dramp.tile([S, P], bf16, name="h_shard")
            h_full = dramp.tile([N, P], bf16, addr_space="Shared",
                                name="h_full")

            with tc.tile_pool(name="const", bufs=1) as cp:
                iota_i = cp.tile([P, P], i32, name="iota_i")
                nc.gpsimd.iota(iota_i[:], pattern=[[1, P]], base=0,
                               channel_multiplier=0)
                iota_f = cp.tile([P, P], f32, name="iota_f")
                nc.vector.tensor_copy(iota_f[:], iota_i[:])
                w1s = cp.tile([P, Rp * H], bf16, name="w1s")
                nc.sync.dma_start(out=w1s[:], in_=w1f[:])
                w2s = cp.tile([P, Rp * O], bf16, name="w2s")
                nc.sync.dma_start(out=w2s[:], in_=w2f[:])
                b1s = cp.tile([P, H], f32, name="b1s")
                nc.sync.dma_start(out=b1s[:], in_=b1b[:])
                b2s = cp.tile([P, O], f32, name="b2s")
                nc.sync.dma_start(out=b2s[:], in_=b2b[:])
                gis = cp.tile([P, T3 * P // 16], i16, name="gis")
                nc.sync.dma_start(out=gis[:], in_=gidxt[:])
                d3s = cp.tile([P, T3], f32, name="d3s")
                nc.sync.dma_start(out=d3s[:], in_=d3t[:])
                hacc1 = cp.tile([P, NT * H], f32, name="hacc1")
                hacc2 = cp.tile([P, NT * O], f32, name="hacc2")

                for layer in (1, 2):
                    Hl = H if layer == 1 else O
                    ws = w1s if layer == 1 else w2s
                    hacc = hacc1 if layer == 1 else hacc2

                    with tc.tile_pool(name=f"sb{layer}", bufs=BUFS) as sb, \
                         tc.tile_pool(name=f"ps{layer}", bufs=3,
                                      space="PSUM") as psp, \
                         tc.tile_pool(name=f"ps2{layer}", bufs=2,
                                      space="PSUM") as ps2:
                        cur = None
                        for (s0, ncols, halfb) in gops:
                            nch = ncols // P
                            gat = sb.tile([P, ncols], bf16, tag="gat",
                                          name="gat")
                            if layer == 1:
                                gbase = feat_bf[halfb * HALF:N, :]
                            else:
                                gbase = h_full[halfb * HALF:N, :]
                            nc.gpsimd.dma_gather(
                                gat[:].rearrange("p (o n) -> p o n", o=1),
                                gbase, gis[:, s0 // 16:(s0 + ncols) // 16],
                                ncols, ncols, P, elem_step=P, transpose=True)
                            selb = sb.tile([P, nch * P], bf16, tag="selb",
                                           name="selb")
                            nc.vector.tensor_tensor(
                                out=selb[:].rearrange("p (g j) -> p g j",
                                                      g=nch),
                                in0=_bc_inner(
                                    d3s[:, s0 // P:s0 // P + nch], P),
                                in1=_bc_mid(iota_f[:], nch),
                                op=AO.is_equal)
                            for b0 in range(0, nch, SCH):
                                nb = min(SCH, nch - b0)
                                msp = psp.tile([P, SCH * Hl], f32,
                                               tag="msp", name="msp")
                                for j in range(nb):
                                    gc = s0 // P + b0 + j
                                    for (a, b, r) in chunk_segs[gc]:
                                        nc.tensor.matmul(
                                            out=msp[a:b,
                                                    j * Hl:(j + 1) * Hl],
                                            lhsT=gat[:, (b0 + j) * P + a:
                                                     (b0 + j) * P + b],
                                            rhs=ws[:, r * Hl:(r + 1) * Hl],
                                            start=True, stop=True)
                                mstg = sb.tile([P, SCH * Hl], bf16,
                                               tag="mstg", name="mstg")
                                nc.scalar.copy(out=mstg[:, 0:nb * Hl],
                                               in_=msp[:, 0:nb * Hl])
                                for j in range(nb):
                                    gc = s0 // P + b0 + j
                                    t = chunk_tile[gc]
                                    k = chunk_k[gc]
                                    if k == 0:
                                        cur = ps2.tile([P, Hl], f32,
                                                       tag="agp", name="agp")
                                    nc.tensor.matmul(
                                        out=cur[:],
                                        lhsT=selb[:, (b0 + j) * P:
                                                  (b0 + j + 1) * P],
                                        rhs=mstg[:, j * Hl:(j + 1) * Hl],
                                        start=(k == 0),
                                        stop=(k == chunk_kt[gc] - 1))
                                    if k != chunk_kt[gc] - 1:
                                        continue
                                    if chunk_half[gc] == 0:
                                        # park lo-half partial in SBUF
                                        nc.scalar.copy(
                                            out=hacc[:, t * Hl:(t + 1) * Hl],
                                            in_=cur[:])
                                        continue
                                    rows = min(P, S - t * P)
                                    if layer == 1:
                                        hb = sb.tile([P, H], f32, tag="hb",
                                                     name="hb")
                                        nc.vector.tensor_tensor(
                                            out=hb[:], in0=cur[:],
                                            in1=hacc[:, t * Hl:(t + 1) * Hl],
                                            op=AO.add)
                                        nc.vector.tensor_tensor(
                                            out=hb[:], in0=hb[:],
                                            in1=b1s[:], op=AO.add)
                                        nc.vector.tensor_scalar_max(
                                            out=hb[:], in0=hb[:],
                                            scalar1=0.0)
                                        hp = sb.tile([P, P], bf16, tag="hp",
                                                     name="hp")
                                        nc.vector.memset(hp[:, H:P], 0.0)
                                        nc.scalar.copy(out=hp[:, 0:H],
                                                       in_=hb[:])
                                        nc.sync.dma_start(
                                            out=h_shard[t * P:t * P + rows,
                                                        :],
                                            in_=hp[:rows, :])
                                    else:
                                        ob = sb.tile([P, O], f32, tag="ob",
                                                     name="ob")
                                        nc.vector.tensor_tensor(
                                            out=ob[:], in0=cur[:],
                                            in1=hacc[:, t * Hl:(t + 1) * Hl],
                                            op=AO.add)
                                        nc.vector.tensor_tensor(
                                            out=ob[:], in0=ob[:],
                                            in1=b2s[:], op=AO.add)
                                        nc.sync.dma_start(
                                            out=outs[t * P:t * P + rows, :],
                                            in_=ob[:rows, :])

                    if layer == 1:
                        if NO_COLLECTIVE:
                            nc.sync.dma_start(out=h_full[0:S, :],
                                              in_=h_shard[:])
                        else:
                            nc.gpsimd.collective_compute(
                                "AllGather", AO.bypass,
                                replica_groups=[list(range(C))],
                                ins=[h_shard[:].opt()],
                                outs=[h_full[:].opt()])

    nc.compile()
    return nc


# ---------------------------------------------------------------- entry

def _run(in_maps, plan, trace=False):
    from concourse.bass_utils import run_bass_kernel_spmd

    key = (plan["T3"], plan["gops"], plan["chunk_segs"], BUFS, GOP)
    nc = _CACHE.get(key)
    if nc is None:
        nc = _build(plan)
        _CACHE[key] = nc
    res = run_bass_kernel_spmd(nc, in_maps, list(range(C)), trace=trace)
    out = np.concatenate([res.results[c]["out_shard"] for c in range(C)],
                         axis=0)
    return out, res


def kernel(**inputs):
    plan, in_maps = _preprocess(**inputs)
    out, _ = _run(in_maps, plan)
    return out


# revision 3
# speedup vs baseline: 1.3404x; 1.3404x over previous
"""2-layer relational GCN (RGCN) on Trainium2, 8-core SPMD — v3.

Sharding: edges partitioned by dst-node range (core c owns dst nodes
[c*S, (c+1)*S)); node features and weights replicated. Self-loops folded
in as relation R.

Single-phase layer structure (no message buffer round trip):
  Slots ordered (src-half, dst-tile, rel). One SWDGE dma_gather stream
  per src-half (int16 index limit) pulls node rows (bf16, 256B) with
  fused transpose, directly usable as matmul lhsT columns. Each
  128-slot chunk belongs to ONE dst tile; its relation segments are
  matmul'd (partition-range PSUM writes) with the per-relation weights,
  staged to bf16, and immediately aggregated into the tile's PSUM
  accumulator with a one-hot dst-selection matmul. The lo-half pass
  parks per-tile partials in SBUF; the hi-half pass adds them back,
  applies bias (+ReLU layer 1), and writes the shard.
  AllGather (bf16, 128-padded h rows) between layers; both layers share
  the same slot order and index tables.

Slot-space structure (group sizes, chunk maps, segment lists, op lists)
is the max over cores so all 8 cores run one program; per-core tables
embed that core's edges (pad slots: gather idx 0, one-hot d = -1).
All SWDGE ops stay on queue 0: concurrent multi-queue SWDGE gathers
lose data on this hardware (empirically; the ring serialization when
ops exceed half the 1024-descriptor carveout is the only stable mode).
"""

import numpy as np

P = 128
C = 8
HALF = 32768
GOP = 896         # slots per gather op (>=1024-desc ops hang the DGE ring)
SCH = 8           # chunks per PSUM bank batch
BUFS = 4
NO_COLLECTIVE = False

_CACHE = {}


def _wrap16(a):
    a = np.asarray(a, np.int16)
    assert len(a) % 16 == 0
    w = a.reshape(-1, 16).T
    return np.ascontiguousarray(np.tile(w, (8, 1)))


# ---------------------------------------------------------------- host prep

def _preprocess(feat, W1, loop1, b1, W2, loop2, b2, src, dst, etype):
    import ml_dtypes

    feat = np.asarray(feat, dtype=np.float32)
    W1 = np.asarray(W1, dtype=np.float32)
    W2 = np.asarray(W2, dtype=np.float32)
    loop1 = np.asarray(loop1, dtype=np.float32)
    loop2 = np.asarray(loop2, dtype=np.float32)
    b1 = np.asarray(b1, dtype=np.float32)
    b2 = np.asarray(b2, dtype=np.float32)
    src = np.asarray(src).astype(np.int64).ravel()
    dst = np.asarray(dst).astype(np.int64).ravel()
    etype = np.asarray(etype).astype(np.int64).ravel()

    N, D = feat.shape
    R, _, H = W1.shape
    O = W2.shape[2]
    assert D == P and N % C == 0
    S = N // C
    NT = -(-S // P)
    Rp = R + 1

    sl = np.arange(N, dtype=np.int64)
    asrc = np.concatenate([src, sl])
    adst = np.concatenate([dst, sl])
    aet = np.concatenate([etype, np.full(N, R, dtype=np.int64)])
    core_of = adst // S

    cores = []
    for c in range(C):
        m = core_of == c
        es, el, er = asrc[m], adst[m] - c * S, aet[m]
        cores.append((es, el, er, el // P, (es >= HALF).astype(np.int64)))

    # common per-(half, tile, rel) max counts (unpadded: the high-to-low
    # segment emission below handles arbitrary group boundaries)
    cnt = np.zeros((2, NT, Rp), np.int64)
    for es, el, er, tid, hf in cores:
        k = (hf * NT + tid) * Rp + er
        cnt = np.maximum(
            cnt, np.bincount(k, minlength=2 * NT * Rp).reshape(2, NT, Rp))

    # slot layout: (half, tile, rel) groups unpadded; (half, tile) blocks
    # padded to 128; both halves concatenated
    ht_size = cnt.sum(axis=2)                        # [2, NT]
    ht_pad = np.maximum(((ht_size + P - 1) // P) * P, P)
    ht_off = np.zeros((2, NT + 1), np.int64)
    base = 0
    half_range = []
    for h in range(2):
        h0 = base
        for t in range(NT):
            ht_off[h, t] = base
            base += int(ht_pad[h, t])
        ht_off[h, NT] = base
        half_range.append((h0, base))
    T3 = base // P
    assert base % P == 0

    # group offsets within each (h, t) block
    goff = np.zeros((2, NT, Rp + 1), np.int64)
    for h in range(2):
        for t in range(NT):
            goff[h, t, 1:] = np.cumsum(cnt[h, t])
            goff[h, t] += ht_off[h, t]

    # chunk maps + per-chunk relation segments
    chunk_tile = np.zeros(T3, np.int64)
    chunk_half = np.zeros(T3, np.int64)
    chunk_k = np.zeros(T3, np.int64)
    chunk_kt = np.zeros(T3, np.int64)
    chunk_segs = []
    for h in range(2):
        for t in range(NT):
            kt = int(ht_pad[h, t]) // P
            c0 = int(ht_off[h, t]) // P
            for k in range(kt):
                gc = c0 + k
                chunk_tile[gc] = t
                chunk_half[gc] = h
                chunk_k[gc] = k
                chunk_kt[gc] = kt
                lo = ht_off[h, t] + k * P
                hi = lo + P
                segs = []
                for r in range(Rp):
                    a = max(int(goff[h, t, r]), int(lo))
                    b = min(int(goff[h, t, r + 1]), int(hi))
                    if b > a:
                        segs.append((int(a - lo), int(b - lo), r))
                if not segs:
                    segs = [(0, P, 0)]
                else:
                    a0, _, r0 = segs[0]
                    if a0 > 0:
                        segs[0] = (0, segs[0][1], r0)
                    an, bn, rn = segs[-1]
                    if bn < P:
                        segs[-1] = (an, P, rn)
                # matmul PSUM partition windows are limited to [0,*),
                # [32,64), [64,*): emit segments high-to-low, each from the
                # largest legal base covering its span; clobbered low rows
                # are rewritten by the following (lower) segments
                segs = sorted(segs, key=lambda x: -x[0])
                fixed = []
                for (a, b, r) in segs:
                    if b <= 64:
                        base_p = 32 if a >= 32 else 0
                    else:
                        base_p = 64 if a >= 64 else 0
                    fixed.append((base_p, b, r))
                chunk_segs.append(tuple(fixed))

    # gather ops: per half, 128-aligned, <= GOP slots
    gops = []
    for h in range(2):
        a, b = half_range[h]
        s0 = a
        while s0 < b:
            n = min(GOP, b - s0)
            gops.append((int(s0), int(n), h))
            s0 += n

    # replicated tensors
    feat_bf = np.ascontiguousarray(feat.astype(ml_dtypes.bfloat16))
    w1f = np.ascontiguousarray(
        np.concatenate([W1, loop1[None]], axis=0)
        .transpose(1, 0, 2).reshape(D, Rp * H).astype(ml_dtypes.bfloat16))
    W2e = np.concatenate([W2, loop2[None]], axis=0)
    w2p = np.zeros((P, Rp * O), np.float32)
    w2p[:H, :] = W2e.transpose(1, 0, 2).reshape(H, Rp * O)
    w2f = np.ascontiguousarray(w2p.astype(ml_dtypes.bfloat16))
    b1b = np.ascontiguousarray(np.broadcast_to(b1, (P, H)).astype(np.float32))
    b2b = np.ascontiguousarray(np.broadcast_to(b2, (P, O)).astype(np.float32))

    # per-core tables
    in_maps = []
    for es, el, er, tid, hf in cores:
        nE = len(es)
        gkey = (hf * NT + tid) * Rp + er
        order = np.lexsort((es, gkey))
        sk = gkey[order]
        uq, ust = np.unique(sk, return_index=True)
        goff_flat = goff[:, :, :Rp].reshape(-1)
        slot_sorted = (goff_flat[sk]
                       + (np.arange(nE) - ust[np.searchsorted(uq, sk)]))
        slot = np.empty(nE, np.int64)
        slot[order] = slot_sorted

        gidx = np.zeros(T3 * P, np.int16)
        gidx[slot] = (es - HALF * hf).astype(np.int16)
        d3 = np.full(T3 * P, -1.0, np.float32)
        d3[slot] = (el % P).astype(np.float32)

        in_maps.append({
            "feat_bf": feat_bf, "w1f": w1f, "w2f": w2f,
            "b1b": b1b, "b2b": b2b,
            "gidx": _wrap16(gidx),
            "d3t": np.ascontiguousarray(d3.reshape(T3, P).T),
        })

    plan = dict(N=N, D=D, H=H, O=O, Rp=Rp, S=S, NT=NT, T3=T3,
                gops=tuple(gops),
                chunk_tile=tuple(int(x) for x in chunk_tile),
                chunk_half=tuple(int(x) for x in chunk_half),
                chunk_k=tuple(int(x) for x in chunk_k),
                chunk_kt=tuple(int(x) for x in chunk_kt),
                chunk_segs=tuple(chunk_segs))
    return plan, in_maps


# ---------------------------------------------------------------- device prog

def _bc_inner(ap, n):
    import concourse.bass as bass
    return bass.AP(ap.tensor, ap.offset, list(ap.ap) + [[0, n]])


def _bc_mid(ap, g):
    import concourse.bass as bass
    a = list(ap.ap)
    return bass.AP(ap.tensor, ap.offset, [a[0], [0, g]] + a[1:])


def _build(plan):
    import concourse.bacc as bacc
    import concourse.tile as tile
    import concourse.mybir as mybir

    N, D, H, O, Rp = plan["N"], plan["D"], plan["H"], plan["O"], plan["Rp"]
    S, NT, T3 = plan["S"], plan["NT"], plan["T3"]
    gops = plan["gops"]
    chunk_tile, chunk_half = plan["chunk_tile"], plan["chunk_half"]
    chunk_k, chunk_kt = plan["chunk_k"], plan["chunk_kt"]
    chunk_segs = plan["chunk_segs"]
    f32 = mybir.dt.float32
    bf16 = mybir.dt.bfloat16
    i16 = mybir.dt.int16
    i32 = mybir.dt.int32
    AO = mybir.AluOpType

    nc = bacc.Bacc("TRN2", target_bir_lowering=False, debug=False,
                   num_devices=C)
    feat_bf = nc.dram_tensor("feat_bf", [N, P], bf16, kind="ExternalInput")
    w1f = nc.dram_tensor("w1f", [P, Rp * H], bf16, kind="ExternalInput")
    w2f = nc.dram_tensor("w2f", [P, Rp * O], bf16, kind="ExternalInput")
    b1b = nc.dram_tensor("b1b", [P, H], f32, kind="ExternalInput")
    b2b = nc.dram_tensor("b2b", [P, O], f32, kind="ExternalInput")
    gidxt = nc.dram_tensor("gidx", [P, T3 * P // 16], i16,
                           kind="ExternalInput")
    d3t = nc.dram_tensor("d3t", [P, T3], f32, kind="ExternalInput")
    outs = nc.dram_tensor("out_shard", [S, O], f32, kind="ExternalOutput")

    with tile.TileContext(nc) as tc:
        with tc.tile_pool(name="dram", bufs=1, space="DRAM") as dramp:
            h_shard = dramp.tile([S, P], bf16, name="h_shard")
            h_full = dramp.tile([N, P], bf16, addr_space="Shared",
                                name="h_full")

            with tc.tile_pool(name="const", bufs=1) as cp:
                iota_i = cp.tile([P, P], i32, name="iota_i")
                nc.gpsimd.iota(iota_i[:], pattern=[[1, P]], base=0,
                               channel_multiplier=0)
                iota_f = cp.tile([P, P], f32, name="iota_f")
                nc.vector.tensor_copy(iota_f[:], iota_i[:])
                w1s = cp.tile([P, Rp * H], bf16, name="w1s")
                nc.sync.dma_start(out=w1s[:], in_=w1f[:])
                w2s = cp.tile([P, Rp * O], bf16, name="w2s")
                nc.sync.dma_start(out=w2s[:], in_=w2f[:])
                b1s = cp.tile([P, H], f32, name="b1s")
                nc.sync.dma_start(out=b1s[:], in_=b1b[:])
                b2s = cp.tile([P, O], f32, name="b2s")
                nc.sync.dma_start(out=b2s[:], in_=b2b[:])
                gis = cp.tile([P, T3 * P // 16], i16, name="gis")
                nc.sync.dma_start(out=gis[:], in_=gidxt[:])
                d3s = cp.tile([P, T3], f32, name="d3s")
                nc.sync.dma_start(out=d3s[:], in_=d3t[:])
                hacc1 = cp.tile([P, NT * H], f32, name="hacc1")
                hacc2 = cp.tile([P, NT * O], f32, name="hacc2")

                for layer in (1, 2):
                    Hl = H if layer == 1 else O
                    ws = w1s if layer == 1 else w2s
                    hacc = hacc1 if layer == 1 else hacc2

                    with tc.tile_pool(name=f"sb{layer}", bufs=BUFS) as sb, \
                         tc.tile_pool(name=f"ps{layer}", bufs=3,
                                      space="PSUM") as psp, \
                         tc.tile_pool(name=f"ps2{layer}", bufs=2,
                                      space="PSUM") as ps2:
                        cur = None
                        for (s0, ncols, halfb) in gops:
                            nch = ncols // P
                            gat = sb.tile([P, ncols], bf16, tag="gat",
                                          name="gat")
                            if layer == 1:
                                gbase = feat_bf[halfb * HALF:N, :]
                            else:
                                gbase = h_full[halfb * HALF:N, :]
                            nc.gpsimd.dma_gather(
                                gat[:].rearrange("p (o n) -> p o n", o=1),
                                gbase, gis[:, s0 // 16:(s0 + ncols) // 16],
                                ncols, ncols, P, elem_step=P, transpose=True)
                            selb = sb.tile([P, nch * P], bf16, tag="selb",
                                           name="selb")
                            nc.vector.tensor_tensor(
                                out=selb[:].rearrange("p (g j) -> p g j",
                                                      g=nch),
                                in0=_bc_inner(
                                    d3s[:, s0 // P:s0 // P + nch], P),
                                in1=_bc_mid(iota_f[:], nch),
                                op=AO.is_equal)
                            for b0 in range(0, nch, SCH):
                                nb = min(SCH, nch - b0)
                                msp = psp.tile([P, SCH * Hl], f32,
                                               tag="msp", name="msp")
                                for j in range(nb):
                                    gc = s0 // P + b0 + j
                                    for (a, b, r) in chunk_segs[gc]:
                                        nc.tensor.matmul(
                                            out=msp[a:b,
                                                    j * Hl:(j + 1) * Hl],
                                            lhsT=gat[:, (b0 + j) * P + a:
                                                     (b0 + j) * P + b],
                                            rhs=ws[:, r * Hl:(r + 1) * Hl],
                                            start=True, stop=True)
                                mstg = sb.tile([P, SCH * Hl], bf16,
                                               tag="mstg", name="mstg")
                                nc.scalar.copy(out=mstg[:, 0:nb * Hl],
                                               in_=msp[:, 0:nb * Hl])
                                for j in range(nb):
                                    gc = s0 // P + b0 + j
                                    t = chunk_tile[gc]
                                    k = chunk_k[gc]
                                    if k == 0:
                                        cur = ps2.tile([P, Hl], f32,
                                                       tag="agp", name="agp")
                                    nc.tensor.matmul(
                                        out=cur[:],
                                        lhsT=selb[:, (b0 + j) * P:
                                                  (b0 + j + 1) * P],
                                        rhs=mstg[:, j * Hl:(j + 1) * Hl],
                                        start=(k == 0),
                                        stop=(k == chunk_kt[gc] - 1))
                                    if k != chunk_kt[gc] - 1:
                                        continue
                                    if chunk_half[gc] == 0:
                                        # park lo-half partial in SBUF
                                        nc.scalar.copy(
                                            out=hacc[:, t * Hl:(t + 1) * Hl],
                                            in_=cur[:])
                                        continue
                                    rows = min(P, S - t * P)
                                    if layer == 1:
                                        hb = sb.tile([P, H], f32, tag="hb",
                                                     name="hb")
                                        nc.vector.tensor_tensor(
                                            out=hb[:], in0=cur[:],
                                            in1=hacc[:, t * Hl:(t + 1) * Hl],
                                            op=AO.add)
                                        nc.vector.tensor_tensor(
                                            out=hb[:], in0=hb[:],
                                            in1=b1s[:], op=AO.add)
                                        nc.vector.tensor_scalar_max(
                                            out=hb[:], in0=hb[:],
                                            scalar1=0.0)
                                        hp = sb.tile([P, P], bf16, tag="hp",
                                                     name="hp")
                                        nc.vector.memset(hp[:, H:P], 0.0)
                                        nc.scalar.copy(out=hp[:, 0:H],
                                                       in_=hb[:])
                                        nc.sync.dma_start(
                                            out=h_shard[t * P:t * P + rows,
                                                        :],
                                            in_=hp[:rows, :])
                                    else:
                                        ob = sb.tile([P, O], f32, tag="ob",
                                                     name="ob")
                                        nc.vector.tensor_tensor(
                                            out=ob[:], in0=cur[:],
                                            in1=hacc[:, t * Hl:(t + 1) * Hl],
                                            op=AO.add)
                                        nc.vector.tensor_tensor(
                                            out=ob[:], in0=ob[:],
                                            in1=b2s[:], op=AO.add)
                                        nc.sync.dma_start(
                                            out=outs[t * P:t * P + rows, :],
                                            in_=ob[:rows, :])

                    if layer == 1:
                        if NO_COLLECTIVE:
                            nc.sync.dma_start(out=h_full[0:S, :],
                                              in_=h_shard[:])
                        else:
                            nc.gpsimd.collective_compute(
                                "AllGather", AO.bypass,
                                replica_groups=[list(range(C))],
                                ins=[h_shard[:].opt()],
                                outs=[h_full[:].opt()])

    nc.compile()
    return nc


# ---------------------------------------------------------------- entry

def _run(in_maps, plan, trace=False):
    from concourse.bass_utils import run_bass_kernel_spmd

    key = (plan["T3"], plan["gops"], plan["chunk_segs"], BUFS, GOP)
    nc = _CACHE.get(key)
    if nc is None:
        nc = _build(plan)
        _CACHE[key] = nc
    res = run_bass_kernel_spmd(nc, in_maps, list(range(C)), trace=trace)
    out = np.concatenate([res.results[c]["out_shard"] for c in range(C)],
                         axis=0)
    return out, res


def kernel(**inputs):
    plan, in_maps = _preprocess(**inputs)
    out, _ = _run(in_maps, plan)
    return out
